# revision 1
# baseline (speedup 1.0000x reference)
# Trainium2 Bass kernel for nn_CustomAttention (fused qkv + LoRA + per-head
# LayerNorm + softmax attention + output projection).
#
# Sharding: 16 heads split across 8 cores (2 heads/core), both batch elements
# on every core. Each core computes its heads' attention and its partial
# output projection (sum over its heads' columns); the host sums the 8
# partials and adds proj_b. LoRA is folded into the qkv weights on the host:
#   x@W.T + (x@A)@B*s == x@(W + s*(A@B).T).T
#
# Per-core layout choices (see comments in _build_program):
#  - scores are computed transposed (sT[j,i]) so softmax-normalized output
#    comes out in [d, i] layout, which is exactly the lhsT the projection
#    matmul needs -> no attention-weight transposes at all.
#  - exp without max subtraction (softmax is shift-invariant; post-LayerNorm
#    scores are bounded by ~|D|^0.5 so fp32 exp cannot overflow).
#  - the attention@v matmul uses stationary [v | ones]: output partitions
#    0-63 hold out^T, partitions 64-127 hold the softmax denominator
#    replicated, so normalization is a reciprocal + one multiply.
import numpy as np
import ml_dtypes

import concourse.bass as bass
import concourse.bacc as bacc
import concourse.mybir as mybir
from concourse.tile import TileContext
from concourse.masks import make_identity
from concourse.bass_utils import run_bass_kernel_spmd

BF16 = ml_dtypes.bfloat16
F32 = np.float32

B, N, DIM, H, R = 2, 2048, 1024, 16, 8
D = DIM // H              # 64
NCORES = 8
HPC = H // NCORES         # 2 heads per core
ALPHA = 8.0
LORA_SCALE = ALPHA / R
EPS = 1e-5
QSCALE = float(D) ** -0.5  # 0.125

NCH = DIM // 128          # 8 contraction chunks of 128
NTI = N // 128            # 16 row tiles of 128
NTI8 = N // 256           # 8 i-tiles of 256
QI = 256                  # query-tile width (free dim of score matmuls)

_prog_cache: dict = {}


def _build_program(use_mask: bool, affine_q: bool, affine_k: bool, repeat: int = 1,
                   phases: str = "AB"):
    nc = bacc.Bacc("TRN2", target_bir_lowering=False)
    f32 = mybir.dt.float32
    bf16 = mybir.dt.bfloat16

    # xT layout: [cm, b, ci, n] so each partition's load is one contiguous
    # 32KB run; out_p layout: [cm, b, ti, c] for 8KB-contiguous stores.
    xT = nc.dram_tensor("xT", [128, B, NCH, N], bf16, kind="ExternalInput")
    wT = nc.dram_tensor("wT", [NCH, 128, 6 * D], bf16, kind="ExternalInput")
    projT = nc.dram_tensor("projT", [D, HPC, DIM], bf16, kind="ExternalInput")
    out_p = nc.dram_tensor("out_p", [128, B, NTI, DIM], f32, kind="ExternalOutput")
    if affine_q or affine_k:
        # rows: 0=qw*scale 1=qb*scale 2=kw 3=kb, each broadcast to 128 parts
        lnaff = nc.dram_tensor("lnaff", [4, 128, D], f32, kind="ExternalInput")
    if use_mask:
        emaskT = nc.dram_tensor("emaskT", [N, N], bf16, kind="ExternalInput")

    with TileContext(nc) as tc:
        import contextlib
        with contextlib.ExitStack() as ctx:
            const = ctx.enter_context(tc.tile_pool(name="const", bufs=1))
            ident = const.tile([128, 128], bf16)
            make_identity(nc, ident)
            eps_t = const.tile([128, 1], f32)
            nc.vector.memset(eps_t, EPS)

            persist = ctx.enter_context(tc.tile_pool(name="persist", bufs=1))
            w_sb = persist.tile([128, NCH, 6 * D], bf16)
            nc.sync.dma_start(out=w_sb, in_=wT.rearrange("ci cm w -> cm ci w"))
            proj_sb = persist.tile([D, HPC, DIM], bf16)
            nc.sync.dma_start(out=proj_sb, in_=projT[:, :, :])
            if affine_q or affine_k:
                aff_sb = persist.tile([128, 4, D], f32)
                nc.sync.dma_start(out=aff_sb, in_=lnaff.rearrange("r p d -> p r d"))

            # per-b persistent activations (rewritten each b; Tile handles WAR)
            xpool = ctx.enter_context(tc.tile_pool(name="xpool", bufs=2))
            qkpool = ctx.enter_context(tc.tile_pool(name="qkpool", bufs=2))
            vpool = ctx.enter_context(tc.tile_pool(name="vpool", bufs=2))

            if repeat > 1:
                ctx.enter_context(tc.For_i(
                    0, repeat, 1,
                    hint_engines=(mybir.EngineType.PE, mybir.EngineType.SP,
                                  mybir.EngineType.Activation,
                                  mybir.EngineType.DVE, mybir.EngineType.Pool)))
            # hoist both batches' input loads so b=1 prefetches under b=0
            x_sbs = []
            for b in range(B):
                x_sb = xpool.tile([128, NCH, N], bf16, tag="x_sb")
                nc.sync.dma_start(out=x_sb[:, 0:NCH // 2, :],
                                  in_=xT[:, b, 0:NCH // 2, :])
                nc.sync.dma_start(out=x_sb[:, NCH // 2:, :],
                                  in_=xT[:, b, NCH // 2:, :])
                x_sbs.append(x_sb)
            for b in range(B):
                x_sb = x_sbs[b]
                # qT/kT: partitions 0-63 head0, 64-127 head1; free = n
                qT_sb = qkpool.tile([128, N], bf16, tag="qT")
                kT_sb = qkpool.tile([128, N], bf16, tag="kT")
                # vplus: [j_mod, chunk, head, 64 v | 64 ones]
                vp_sb = vpool.tile([128, NTI, HPC, 128], bf16, tag="vp")
                nc.vector.memset(vp_sb[:, :, :, D:], 1.0)
                if "A" not in phases:  # timing variant: fill A outputs
                    nc.vector.memset(qT_sb, 0.5)
                    nc.vector.memset(kT_sb, 0.5)
                    nc.vector.memset(vp_sb[:, :, :, :D], 0.5)

                # ---------------- phase A: qkv gen + LN + transposes --------
                # qkv staged to SBUF f32; LayerNorm stats batched across all
                # 16 row-tiles x 4 instances into a few large ops.
                if "A" not in phases:
                    pass
                else:
                 with tc.tile_pool(name="psA", bufs=2, space="PSUM") as psA, \
                     tc.tile_pool(name="psT", bufs=2, space="PSUM") as psT, \
                     tc.tile_pool(name="stg", bufs=1) as stg, \
                     tc.tile_pool(name="lnp", bufs=2) as lnp, \
                     tc.tile_pool(name="natp", bufs=8) as natp:
                    stage = stg.tile([128, NTI, 6 * D], f32, tag="stage")
                    sqs = stg.tile([128, NTI, 6 * D], f32, tag="sqs")
                    for ti in range(NTI):
                        pq = psA.tile([128, 6 * D], f32, tag="pq")
                        for ci in range(NCH):
                            nc.tensor.matmul(
                                pq,
                                lhsT=x_sb[:, ci, ti * 128:(ti + 1) * 128],
                                rhs=w_sb[:, ci, :],
                                start=(ci == 0),
                                stop=(ci == NCH - 1),
                            )
                        nc.scalar.copy(out=stage[:, ti, :], in_=pq)
                        nc.vector.tensor_tensor(
                            out=sqs[:, ti, :], in0=stage[:, ti, :],
                            in1=stage[:, ti, :], op=mybir.AluOpType.mult)
                    # batched stats, in ti-halves so stats(h0) overlaps
                    # qkv matmuls of h1: [128, HT, 6, D] -> [128, HT*6]
                    st6v = stage.rearrange("p t (i d) -> p t i d", d=D)
                    sq6v = sqs.rearrange("p t (i d) -> p t i d", d=D)
                    HT = NTI // 2
                    insts = [(0, 0, 0), (1, 1, 0), (3, 0, 1), (4, 1, 1)]
                    for half in range(2):
                     hsl = slice(half * HT, (half + 1) * HT)
                     mean = lnp.tile([128, HT, 6], f32, tag="mean")
                     nc.vector.tensor_reduce(
                        out=mean, in_=st6v[:, hsl], axis=mybir.AxisListType.X,
                        op=mybir.AluOpType.add)
                     nc.vector.tensor_scalar(
                        out=mean, in0=mean, scalar1=1.0 / D, scalar2=None,
                        op0=mybir.AluOpType.mult)
                     var = lnp.tile([128, HT, 6], f32, tag="var")
                     nc.vector.tensor_reduce(
                        out=var, in_=sq6v[:, hsl], axis=mybir.AxisListType.X,
                        op=mybir.AluOpType.add)
                     nc.vector.tensor_scalar(
                        out=var, in0=var, scalar1=1.0 / D, scalar2=None,
                        op0=mybir.AluOpType.mult)
                     m2 = lnp.tile([128, HT, 6], f32, tag="m2")
                     nc.vector.tensor_tensor(
                        out=m2, in0=mean, in1=mean, op=mybir.AluOpType.mult)
                     nc.vector.tensor_tensor(
                        out=var, in0=var, in1=m2, op=mybir.AluOpType.subtract)
                     rstd = lnp.tile([128, HT, 6], f32, tag="rstd")
                     nc.scalar.activation(
                        out=rstd, in_=var,
                        func=mybir.ActivationFunctionType.Sqrt,
                        bias=eps_t, scale=1.0)
                     nc.vector.reciprocal(out=rstd, in_=rstd)
                     if not affine_q:  # fold q scaling (D^-0.5) into rstd
                        nc.vector.tensor_scalar(
                            out=rstd[:, :, 0:6:3], in0=rstd[:, :, 0:6:3],
                            scalar1=QSCALE, scalar2=None,
                            op0=mybir.AluOpType.mult)
                     for tih in range(HT):
                        ti = half * HT + tih
                        pt = psT.tile([128, 2, 128], bf16, tag="pt")
                        for inst, qk, hh in insts:
                            affine = affine_q if qk == 0 else affine_k
                            nat = natp.tile([128, D], bf16, tag="nat")
                            if affine:
                                natf = natp.tile([128, D], f32, tag="natf")
                                nc.vector.tensor_scalar(
                                    out=natf, in0=st6v[:, ti, inst, :],
                                    scalar1=mean[:, tih, inst:inst + 1],
                                    scalar2=rstd[:, tih, inst:inst + 1],
                                    op0=mybir.AluOpType.subtract,
                                    op1=mybir.AluOpType.mult)
                                r = 0 if qk == 0 else 2
                                natf2 = natp.tile([128, D], f32, tag="natf2")
                                nc.vector.tensor_tensor(
                                    out=natf2, in0=natf, in1=aff_sb[:, r, :],
                                    op=mybir.AluOpType.mult)
                                nc.vector.tensor_tensor(
                                    out=nat, in0=natf2, in1=aff_sb[:, r + 1, :],
                                    op=mybir.AluOpType.add)
                            else:
                                nc.vector.tensor_scalar(
                                    out=nat, in0=st6v[:, ti, inst, :],
                                    scalar1=mean[:, tih, inst:inst + 1],
                                    scalar2=rstd[:, tih, inst:inst + 1],
                                    op0=mybir.AluOpType.subtract,
                                    op1=mybir.AluOpType.mult)
                            nc.tensor.transpose(
                                pt[hh * D:(hh + 1) * D, qk, :], nat, ident)
                        nc.scalar.copy(
                            out=qT_sb[:, ti * 128:(ti + 1) * 128], in_=pt[:, 0, :])
                        nc.scalar.copy(
                            out=kT_sb[:, ti * 128:(ti + 1) * 128], in_=pt[:, 1, :])
                        nc.gpsimd.tensor_copy(
                            out=vp_sb[:, ti, :, 0:D],
                            in_=stage.rearrange("p t (h x) -> p t h x", h=2)
                                [:, ti, :, 2 * D:3 * D])

                # ---------------- phase B: attention + projection -----------
                if "B" not in phases:
                    pass
                else:
                 with tc.tile_pool(name="psS", bufs=2, space="PSUM") as psS, \
                     tc.tile_pool(name="psAV", bufs=2, space="PSUM") as psAV, \
                     tc.tile_pool(name="psP", bufs=1, space="PSUM") as psP, \
                     tc.tile_pool(name="esp", bufs=2) as esp, \
                     tc.tile_pool(name="otp", bufs=3) as otp, \
                     tc.tile_pool(name="outp", bufs=2) as outp, \
                     tc.tile_pool(name="mskp", bufs=2) as mskp:
                    for ti8 in range(NTI8):
                        i0 = ti8 * QI
                        oTs = []
                        for hh in range(HPC):
                            hs = slice(hh * D, (hh + 1) * D)
                            av = psAV.tile([128, QI], f32, tag="av")
                            for jq in range(4):
                                sT = psS.tile([128, 4, QI], f32, tag="sT")
                                for cj in range(4):
                                    j = jq * 4 + cj
                                    nc.tensor.matmul(
                                        sT[:, cj, :],
                                        lhsT=kT_sb[hs, j * 128:(j + 1) * 128],
                                        rhs=qT_sb[hs, i0:i0 + QI],
                                        start=True, stop=True,
                                    )
                                es = esp.tile([128, 4, QI], bf16, tag="es")
                                nc.scalar.activation(
                                    out=es, in_=sT,
                                    func=mybir.ActivationFunctionType.Exp,
                                )
                                if use_mask:
                                    msk = mskp.tile([128, 4, QI], bf16, tag="msk")
                                    for cj in range(4):
                                        j = jq * 4 + cj
                                        nc.sync.dma_start(
                                            out=msk[:, cj, :],
                                            in_=emaskT[j * 128:(j + 1) * 128,
                                                       i0:i0 + QI],
                                        )
                                    nc.vector.tensor_tensor(
                                        out=es, in0=es, in1=msk,
                                        op=mybir.AluOpType.mult,
                                    )
                                for cj in range(4):
                                    j = jq * 4 + cj
                                    nc.tensor.matmul(
                                        av,
                                        lhsT=vp_sb[:, j, hh, :],
                                        rhs=es[:, cj, :],
                                        start=(j == 0), stop=(j == NTI - 1),
                                    )
                            zr = otp.tile([D, QI], f32, tag="zr")
                            nc.vector.reciprocal(out=zr, in_=av[D:, :])
                            oT = otp.tile([D, QI], bf16, tag="oT")
                            nc.vector.tensor_tensor(
                                out=oT, in0=av[0:D, :], in1=zr,
                                op=mybir.AluOpType.mult,
                            )
                            oTs.append(oT)
                        osb = outp.tile([128, QI // 128, DIM], f32, tag="osb")
                        for sub in range(QI // 128):
                            pp = psP.tile([128, DIM], f32, tag="pp")
                            for nh in range(2):
                                for hh in range(HPC):
                                    nc.tensor.matmul(
                                        pp[:, nh * 512:(nh + 1) * 512],
                                        lhsT=oTs[hh][:, sub * 128:(sub + 1) * 128],
                                        rhs=proj_sb[:, hh, nh * 512:(nh + 1) * 512],
                                        start=(hh == 0), stop=(hh == HPC - 1),
                                    )
                            nc.vector.tensor_copy(out=osb[:, sub, :], in_=pp)
                        ti0 = ti8 * (QI // 128)
                        nc.scalar.dma_start(
                            out=out_p[:, b, ti0:ti0 + QI // 128, :], in_=osb)
    nc.compile()
    return nc


def _prep_inputs(inputs):
    x = np.ascontiguousarray(inputs["x"], dtype=F32)
    qkv_w = np.asarray(inputs["qkv_w"], dtype=F32)
    proj_w = np.asarray(inputs["proj_w"], dtype=F32)
    W_eff = qkv_w.copy()
    for i, (a, bm) in enumerate([("lora_Aq", "lora_Bq"), ("lora_Ak", "lora_Bk"),
                                 ("lora_Av", "lora_Bv")]):
        A = np.asarray(inputs[a], dtype=F32)
        Bm = np.asarray(inputs[bm], dtype=F32)
        W_eff[i * DIM:(i + 1) * DIM] += LORA_SCALE * (A @ Bm).T

    # [cm, b, ci, n] with cm = c % 128, ci = c // 128
    xT_all = np.ascontiguousarray(
        x.transpose(2, 0, 1).reshape(NCH, 128, B, N)
        .transpose(1, 2, 0, 3).astype(BF16))

    qn_w = np.asarray(inputs["qn_w"], F32); qn_b = np.asarray(inputs["qn_b"], F32)
    kn_w = np.asarray(inputs["kn_w"], F32); kn_b = np.asarray(inputs["kn_b"], F32)
    affine_q = not (np.all(qn_w == 1.0) and np.all(qn_b == 0.0))
    affine_k = not (np.all(kn_w == 1.0) and np.all(kn_b == 0.0))
    mask = np.asarray(inputs["attn_mask"], F32)
    use_mask = bool(np.any(mask))

    common = {"xT": xT_all}
    if affine_q or affine_k:
        aff = np.stack([
            np.broadcast_to(qn_w * QSCALE, (128, D)),
            np.broadcast_to(qn_b * QSCALE, (128, D)),
            np.broadcast_to(kn_w, (128, D)),
            np.broadcast_to(kn_b, (128, D)),
        ]).astype(F32)
        common["lnaff"] = np.ascontiguousarray(aff)
    if use_mask:
        common["emaskT"] = np.ascontiguousarray(
            np.exp(mask[0, 0].T).astype(BF16))

    in_maps = []
    for c in range(NCORES):
        h0 = c * HPC
        blocks = []
        for hh in range(HPC):
            h = h0 + hh
            for part in range(3):  # q, k, v
                blocks.append(W_eff[part * DIM + h * D: part * DIM + (h + 1) * D])
        Wlocal = np.concatenate(blocks, axis=0)          # [384, 1024]
        wT_c = np.ascontiguousarray(
            Wlocal.T.reshape(NCH, 128, 6 * D).astype(BF16))
        projT_c = np.ascontiguousarray(np.stack(
            [proj_w[:, (h0 + hh) * D:(h0 + hh + 1) * D].T for hh in range(HPC)],
            axis=1).astype(BF16))                        # [64, 2, 1024]
        m = dict(common)
        m["wT"] = wT_c
        m["projT"] = projT_c
        in_maps.append(m)
    return in_maps, (use_mask, affine_q, affine_k)


def _run(inputs, trace=False):
    in_maps, key = _prep_inputs(inputs)
    if key not in _prog_cache:
        _prog_cache[key] = _build_program(*key)
    nc = _prog_cache[key]
    res = run_bass_kernel_spmd(nc, in_maps, core_ids=list(range(NCORES)),
                               trace=trace)
    acc = np.zeros((128, B, NTI, DIM), dtype=F32)
    for r in res.results:
        acc += r["out_p"]
    # [cm, b, ti, c] -> [b, ti*128+cm, c]
    out = np.ascontiguousarray(acc.transpose(1, 2, 0, 3).reshape(B, N, DIM))
    out += np.asarray(inputs["proj_b"], F32)
    return out, res


def kernel(**inputs) -> np.ndarray:
    out, _ = _run(inputs)
    return out



# revision 12
# speedup vs baseline: 2.2347x; 2.2347x over previous
# Trainium2 Bass kernel for nn_CustomAttention (fused qkv + LoRA + per-head
# LayerNorm + softmax attention + output projection).
#
# Sharding: 16 heads split across 8 cores (2 heads/core), both batch elements
# on every core. Each core computes its heads' attention and its partial
# output projection (sum over its heads' columns); the host sums the 8
# partials and adds proj_b. LoRA is folded into the qkv weights on the host:
#   x@W.T + (x@A)@B*s == x@(W + s*(A@B).T).T
# LN mean-removal for q/k is also folded into the weights on the host
# (mean_d q = x @ rowmean(Wq_head)), so the device only needs second moments.
#
# Fast path (no mask, identity LN affine — the graded configuration):
#  - scores and attention@v run in fp8 (e4m3) with MatmulPerfMode.DoubleRow:
#    2 contraction rows per partition -> 2x PE throughput. q/k are stored as
#    [32, 2, N] (d = blk*32 + p), v as [j%128, jtile, head, v|ones].
#  - QSCALE (D^-0.5) is folded into the exp: es = exp(0.125 * s).
#  - rstd = exp(-0.5*ln(ss + 64*eps) + ln(8)) via Act Ln+Exp — both live in
#    the same activation table set as the softmax Exp, so no table reloads.
#  - softmax denominator via stationary [v | ones]: av rows 0-63 hold out^T,
#    rows 64-127 the denominator; one DVE divide produces projected lhsT.
#  - phase interleaving: R0 = qkv+LN+transpose b0; R1 = attention b0
#    interleaved with qkv+LN+transpose b1 (keeps Act busy with exps while PE
#    does b1 qkv); R2 = attention b1 + both batches' output projections.
#    PSUM bank budget (8 banks): R0 psA0(2)+pt0(2); R1 psS(4)+psAV(1)+
#    psA1(1)+pt1(2)=8; R2 psS(4)+psAV(1)+psP(3)=8.
import numpy as np
import ml_dtypes

import concourse.bass as bass
import concourse.bacc as bacc
import concourse.mybir as mybir
from concourse.tile import TileContext
from concourse.masks import make_identity
from concourse.bass_utils import run_bass_kernel_spmd

BF16 = ml_dtypes.bfloat16
F32 = np.float32

B, N, DIM, H, R = 2, 2048, 1024, 16, 8
D = DIM // H              # 64
NCORES = 8
HPC = H // NCORES         # 2 heads per core
ALPHA = 8.0
LORA_SCALE = ALPHA / R
EPS = 1e-5
QSCALE = float(D) ** -0.5  # 0.125

NCH = DIM // 128          # 8 contraction chunks of 128
NTI = N // 128            # 16 row tiles of 128
NTI8 = N // 256           # 8 i-tiles of 256
QI = 256                  # query-tile width (free dim of score matmuls)

_prog_cache: dict = {}


def _build_fast(repeat: int = 1):
    nc = bacc.Bacc("TRN2", target_bir_lowering=False)
    f32 = mybir.dt.float32
    bf16 = mybir.dt.bfloat16
    fp8 = mybir.dt.float8e4
    DR = mybir.MatmulPerfMode.DoubleRow
    Exp = mybir.ActivationFunctionType.Exp
    Ln = mybir.ActivationFunctionType.Ln

    xT = nc.dram_tensor("xT", [128, B, NCH, N], bf16, kind="ExternalInput")
    wT = nc.dram_tensor("wT", [NCH, 128, 6 * D], bf16, kind="ExternalInput")
    projT = nc.dram_tensor("projT", [128, DIM], bf16, kind="ExternalInput")
    out_p = nc.dram_tensor("out_p", [128, B, NTI, DIM], bf16, kind="ExternalOutput")

    with TileContext(nc) as tc:
        import contextlib
        with contextlib.ExitStack() as ctx:
            const = ctx.enter_context(tc.tile_pool(name="const", bufs=1))
            ident8 = const.tile([128, 128], fp8)
            make_identity(nc, ident8)
            beps = const.tile([128, 1], f32)
            nc.vector.memset(beps, 64.0 * EPS)
            bln8 = const.tile([128, 1], f32)
            nc.vector.memset(bln8, float(np.log(8.0)))

            persist = ctx.enter_context(tc.tile_pool(name="persist", bufs=1))
            w_sb = persist.tile([128, NCH, 6 * D], bf16)
            nc.sync.dma_start(out=w_sb, in_=wT.rearrange("ci cm w -> cm ci w"))
            proj_sb = persist.tile([128, DIM], bf16)
            nc.sync.dma_start(out=proj_sb, in_=projT[:, :])

            xpool = ctx.enter_context(tc.tile_pool(name="xpool", bufs=2))
            qk8p = ctx.enter_context(tc.tile_pool(name="qk8p", bufs=2))
            v8p = ctx.enter_context(tc.tile_pool(name="v8p", bufs=2))
            oTp = ctx.enter_context(tc.tile_pool(name="oTp", bufs=2))
            stg = ctx.enter_context(tc.tile_pool(name="stg", bufs=18))
            sqp = ctx.enter_context(tc.tile_pool(name="sqp", bufs=3))
            lnp = ctx.enter_context(tc.tile_pool(name="lnp", bufs=2))
            natp = ctx.enter_context(tc.tile_pool(name="natp", bufs=6))
            esp = ctx.enter_context(tc.tile_pool(name="esp", bufs=3))
            osbp = ctx.enter_context(tc.tile_pool(name="osbp", bufs=4))

            if repeat > 1:
                ctx.enter_context(tc.For_i(
                    0, repeat, 1,
                    hint_engines=(mybir.EngineType.PE, mybir.EngineType.SP,
                                  mybir.EngineType.Activation,
                                  mybir.EngineType.DVE, mybir.EngineType.Pool)))

            # ---- per-iteration activations -------------------------------
            x_sbs, qT8s, kT8s, vp8s, oT2s = [], [], [], [], []
            for b in range(B):
                x_sb = xpool.tile([128, NCH, N], bf16, tag="x_sb")
                nc.sync.dma_start(out=x_sb[:, 0:NCH // 2, :],
                                  in_=xT[:, b, 0:NCH // 2, :])
                nc.sync.dma_start(out=x_sb[:, NCH // 2:, :],
                                  in_=xT[:, b, NCH // 2:, :])
                x_sbs.append(x_sb)
            for b in range(B):
                qT8 = qk8p.tile([64, 2, N], fp8, tag="qT8")
                kT8 = qk8p.tile([64, 2, N], fp8, tag="kT8")
                qT8s.append(qT8)
                kT8s.append(kT8)
                vp8 = v8p.tile([128, NTI, HPC, 128], fp8, tag="vp8")
                nc.vector.memset(vp8[:, :, :, D:], 1.0)
                vp8s.append(vp8)
                oT2 = oTp.tile([128, NTI8, QI], bf16, tag="oT2")
                oT2s.append(oT2)

            # state carried across a_qkv / rstd_calc / a_fin
            a_state: dict = {}

            def a_qkv(b, ti, psA):
                """qkv matmuls + LN second moments for row-tile ti."""
                x_sb = x_sbs[b]
                pq = psA.tile([128, 6 * D], f32, tag="pq")
                for ci in range(NCH):
                    nc.tensor.matmul(
                        pq,
                        lhsT=x_sb[:, ci, ti * 128:(ti + 1) * 128],
                        rhs=w_sb[:, ci, :],
                        start=(ci == 0),
                        stop=(ci == NCH - 1),
                    )
                st = stg.tile([128, 6, D], bf16, tag="st")
                nc.gpsimd.tensor_copy(
                    out=st, in_=pq.rearrange("p (i d) -> p i d", d=D))
                a_state[(b, "st", ti)] = st
                # squares of q/k instances only: [128, head(2), qk(2), D]
                qkv_view = st.rearrange("p i d -> p (i d)") \
                    .rearrange("p (h r) -> p h r", h=2) \
                    .rearrange("p h (i d) -> p h i d", d=D)[:, :, 0:2, :]
                sq = sqp.tile([128, 2, 2, D], f32, tag="sq")
                nc.vector.tensor_tensor(
                    out=sq, in0=qkv_view, in1=qkv_view,
                    op=mybir.AluOpType.mult)
                if ti == 0:
                    ss = lnp.tile([128, NTI, 4], f32, tag="ssall")
                    a_state[(b, "ss")] = ss
                else:
                    ss = a_state[(b, "ss")]
                nc.vector.tensor_reduce(
                    out=ss[:, ti, :],
                    in_=sq, axis=mybir.AxisListType.X,
                    op=mybir.AluOpType.add)

            def rstd_calc(b, half=None):
                """rsqrt(ss/64+eps) = exp(-0.5*ln(ss + 64 eps) + ln(8)).
                Clustered so the Ln<->Exp table swap happens once, not per
                tile."""
                ss = a_state[(b, "ss")]
                sl = slice(0, NTI) if half is None else \
                    slice(half * (NTI // 2), (half + 1) * (NTI // 2))
                nt = NTI if half is None else NTI // 2
                if (b, "rstd") not in a_state:
                    rstdall = lnp.tile([128, NTI, 4], f32, tag="rstdall")
                    a_state[(b, "rstd")] = rstdall
                rstd = a_state[(b, "rstd")]
                tln = lnp.tile([128, nt * 4], f32, tag="tln")
                nc.scalar.activation(
                    out=tln, in_=ss[:, sl, :].rearrange("p a b -> p (a b)"),
                    func=Ln, bias=beps, scale=1.0)
                nc.scalar.activation(
                    out=rstd[:, sl, :].rearrange("p a b -> p (a b)"),
                    in_=tln, func=Exp, scale=-0.5, bias=bln8)

            def a_fin(b, ti, pt):
                """normalize -> fp8 transpose -> qT8/kT8/vp8 copies."""
                st_h = a_state.pop((b, "st", ti))
                rstd = a_state[(b, "rstd")]
                ptt = pt.tile([64, 2, 2, 128], fp8, tag="pt")
                for idx, (inst, qki, hh) in enumerate(
                        [(0, 0, 0), (1, 1, 0), (3, 0, 1), (4, 1, 1)]):
                    nat = natp.tile([128, D], fp8, tag="nat")
                    nc.vector.tensor_scalar(
                        out=nat, in0=st_h[:, inst, :],
                        scalar1=rstd[:, ti, idx:idx + 1],
                        scalar2=None, op0=mybir.AluOpType.mult)
                    for blk in range(2):
                        nc.tensor.transpose(
                            ptt[hh * 32:(hh + 1) * 32, qki, blk, :],
                            nat[:, blk * 32:(blk + 1) * 32], ident8)
                nc.gpsimd.tensor_copy(
                    out=qT8s[b][:, :, ti * 128:(ti + 1) * 128],
                    in_=ptt[:, 0, :, :])
                nc.gpsimd.tensor_copy(
                    out=kT8s[b][:, :, ti * 128:(ti + 1) * 128],
                    in_=ptt[:, 1, :, :])
                nc.gpsimd.tensor_copy(
                    out=vp8s[b][:, ti, :, 0:D],
                    in_=st_h[:, 2:6:3, :])

            def b_block(b, ti8, hh, psS, psAV):
                i0 = ti8 * QI
                av = psAV.tile([128, QI], f32, tag="av")
                for g in range(4):
                    sT = psS.tile([128, 4, QI], f32, tag="sT")
                    for cj in range(4):
                        j = g * 4 + cj
                        nc.tensor.matmul(
                            sT[:, cj, :],
                            lhsT=kT8s[b][hh * 32:(hh + 1) * 32, :,
                                         j * 128:(j + 1) * 128],
                            rhs=qT8s[b][hh * 32:(hh + 1) * 32, :, i0:i0 + QI],
                            start=True, stop=True, perf_mode=DR)
                    es = esp.tile([128, 4, QI], fp8, tag="es")
                    nc.scalar.activation(out=es, in_=sT, func=Exp,
                                         scale=QSCALE)
                    for q2 in range(2):
                        j0 = g * 4 + 2 * q2
                        nc.tensor.matmul(
                            av,
                            lhsT=vp8s[b][:, j0:j0 + 2, hh, :],
                            rhs=es[:, 2 * q2:2 * q2 + 2, :],
                            start=(g == 0 and q2 == 0),
                            stop=(g == 3 and q2 == 1), perf_mode=DR)
                nc.vector.tensor_tensor(
                    out=oT2s[b][hh * D:(hh + 1) * D, ti8, :],
                    in0=av[0:D, :], in1=av[D:, :],
                    op=mybir.AluOpType.divide)

            def proj_chunk(b, ti8, psP, alt):
                for sub in range(2):
                    for nh in range(2):
                        pp = psP.tile([128, 512], f32, tag="pp")
                        nc.tensor.matmul(
                            pp,
                            lhsT=oT2s[b][:, ti8, sub * 128:(sub + 1) * 128],
                            rhs=proj_sb[:, nh * 512:(nh + 1) * 512],
                            start=True, stop=True)
                        osb = osbp.tile([128, 512], bf16, tag="osb")
                        eng = nc.vector if (alt + sub * 2 + nh) % 2 == 0 \
                            else nc.gpsimd
                        eng.tensor_copy(out=osb, in_=pp)
                        nc.sync.dma_start(
                            out=out_p[:, b, ti8 * 2 + sub,
                                      nh * 512:(nh + 1) * 512],
                            in_=osb)

            # ---------------- R0: phase A for b0 --------------------------
            # rstd clustered per half so Ln/Exp table swaps happen while the
            # Act engine is otherwise idle; qkv of the second half overlaps
            # the finishes of the first.
            with tc.tile_pool(name="psA0", bufs=2, space="PSUM") as psA0, \
                 tc.tile_pool(name="pt0", bufs=2, space="PSUM") as pt0:
                for ti in range(NTI // 2):
                    a_qkv(0, ti, psA0)
                rstd_calc(0, half=0)
                for ti in range(NTI // 2, NTI):
                    a_qkv(0, ti, psA0)
                    a_fin(0, ti - NTI // 2, pt0)
                rstd_calc(0, half=1)
                for ti in range(NTI // 2, NTI):
                    a_fin(0, ti, pt0)

            # ---------------- R1 + R2 -------------------------------------
            with tc.tile_pool(name="psS", bufs=2, space="PSUM") as psS, \
                 tc.tile_pool(name="psAV", bufs=1, space="PSUM") as psAV:
                with tc.tile_pool(name="psA1", bufs=1, space="PSUM") as psA1, \
                     tc.tile_pool(name="pt1", bufs=2, space="PSUM") as pt1:
                    for k in range(16):
                        b_block(0, k // 2, k % 2, psS, psAV)
                        if k < 8:
                            a_qkv(1, 2 * k, psA1)
                            a_qkv(1, 2 * k + 1, psA1)
                        else:
                            if k == 8:
                                rstd_calc(1)
                            a_fin(1, 2 * (k - 8), pt1)
                            a_fin(1, 2 * (k - 8) + 1, pt1)
                with tc.tile_pool(name="psP", bufs=3, space="PSUM") as psP:
                    for k in range(16):
                        b_block(1, k // 2, k % 2, psS, psAV)
                        if k % 2 == 0:
                            proj_chunk(0, k // 2, psP, alt=0)
                        elif k >= 3:
                            proj_chunk(1, (k - 3) // 2, psP, alt=1)
                    proj_chunk(1, 7, psP, alt=1)
    nc.compile()
    return nc


def _prep_fast(inputs):
    x = np.ascontiguousarray(inputs["x"], dtype=F32)
    qkv_w = np.asarray(inputs["qkv_w"], dtype=F32)
    proj_w = np.asarray(inputs["proj_w"], dtype=F32)
    W_eff = qkv_w.copy()
    for i, (a, bm) in enumerate([("lora_Aq", "lora_Bq"), ("lora_Ak", "lora_Bk"),
                                 ("lora_Av", "lora_Bv")]):
        A = np.asarray(inputs[a], dtype=F32)
        Bm = np.asarray(inputs[bm], dtype=F32)
        W_eff[i * DIM:(i + 1) * DIM] += LORA_SCALE * (A @ Bm).T
    # fold LN mean removal into q/k weights (per head row-block)
    for part in range(2):
        for h in range(H):
            blk = W_eff[part * DIM + h * D: part * DIM + (h + 1) * D]
            blk -= blk.mean(axis=0, keepdims=True)

    xT_all = np.ascontiguousarray(
        x.transpose(2, 0, 1).reshape(NCH, 128, B, N)
        .transpose(1, 2, 0, 3).astype(BF16))

    in_maps = []
    for c in range(NCORES):
        h0 = c * HPC
        blocks = []
        for hh in range(HPC):
            h = h0 + hh
            for part in range(3):  # q, k, v
                blocks.append(W_eff[part * DIM + h * D: part * DIM + (h + 1) * D])
        Wlocal = np.concatenate(blocks, axis=0)          # [384, 1024]
        wT_c = np.ascontiguousarray(
            Wlocal.T.reshape(NCH, 128, 6 * D).astype(BF16))
        projT_c = np.ascontiguousarray(np.concatenate(
            [proj_w[:, (h0 + hh) * D:(h0 + hh + 1) * D].T for hh in range(HPC)],
            axis=0).astype(BF16))                        # [128, 1024]
        in_maps.append({"xT": xT_all, "wT": wT_c, "projT": projT_c})
    return in_maps


def _flags(inputs):
    qn_w = np.asarray(inputs["qn_w"], F32); qn_b = np.asarray(inputs["qn_b"], F32)
    kn_w = np.asarray(inputs["kn_w"], F32); kn_b = np.asarray(inputs["kn_b"], F32)
    affine_q = not (np.all(qn_w == 1.0) and np.all(qn_b == 0.0))
    affine_k = not (np.all(kn_w == 1.0) and np.all(kn_b == 0.0))
    mask = np.asarray(inputs["attn_mask"], F32)
    use_mask = bool(np.any(mask))
    return use_mask, affine_q, affine_k


def _run(inputs, trace=False):
    key = _flags(inputs)
    if any(key):
        return _run_legacy(inputs, key, trace)
    if ("fast", 1) not in _prog_cache:
        _prog_cache[("fast", 1)] = _build_fast(repeat=1)
    nc = _prog_cache[("fast", 1)]
    in_maps = _prep_fast(inputs)
    res = run_bass_kernel_spmd(nc, in_maps, core_ids=list(range(NCORES)),
                               trace=trace)
    acc = np.zeros((128, B, NTI, DIM), dtype=F32)
    for r in res.results:
        acc += np.asarray(r["out_p"], dtype=F32)
    out = np.ascontiguousarray(acc.transpose(1, 2, 0, 3).reshape(B, N, DIM))
    out += np.asarray(inputs["proj_b"], F32)
    return out, res


def kernel(**inputs) -> np.ndarray:
    out, _ = _run(inputs)
    return out


# ======================= legacy path (mask / affine LN) ====================

def _build_legacy(use_mask: bool, affine_q: bool, affine_k: bool,
                  repeat: int = 1):
    nc = bacc.Bacc("TRN2", target_bir_lowering=False)
    f32 = mybir.dt.float32
    bf16 = mybir.dt.bfloat16

    xT = nc.dram_tensor("xT", [128, B, NCH, N], bf16, kind="ExternalInput")
    wT = nc.dram_tensor("wT", [NCH, 128, 6 * D], bf16, kind="ExternalInput")
    projT = nc.dram_tensor("projT", [D, HPC, DIM], bf16, kind="ExternalInput")
    out_p = nc.dram_tensor("out_p", [128, B, NTI, DIM], f32, kind="ExternalOutput")
    if affine_q or affine_k:
        lnaff = nc.dram_tensor("lnaff", [4, 128, D], f32, kind="ExternalInput")
    if use_mask:
        emaskT = nc.dram_tensor("emaskT", [N, N], bf16, kind="ExternalInput")

    with TileContext(nc) as tc:
        import contextlib
        with contextlib.ExitStack() as ctx:
            const = ctx.enter_context(tc.tile_pool(name="const", bufs=1))
            ident = const.tile([128, 128], bf16)
            make_identity(nc, ident)
            eps_t = const.tile([128, 1], f32)
            nc.vector.memset(eps_t, EPS)

            persist = ctx.enter_context(tc.tile_pool(name="persist", bufs=1))
            w_sb = persist.tile([128, NCH, 6 * D], bf16)
            nc.sync.dma_start(out=w_sb, in_=wT.rearrange("ci cm w -> cm ci w"))
            proj_sb = persist.tile([D, HPC, DIM], bf16)
            nc.sync.dma_start(out=proj_sb, in_=projT[:, :, :])
            if affine_q or affine_k:
                aff_sb = persist.tile([128, 4, D], f32)
                nc.sync.dma_start(out=aff_sb, in_=lnaff.rearrange("r p d -> p r d"))

            xpool = ctx.enter_context(tc.tile_pool(name="xpool", bufs=2))
            qkpool = ctx.enter_context(tc.tile_pool(name="qkpool", bufs=2))
            vpool = ctx.enter_context(tc.tile_pool(name="vpool", bufs=2))

            if repeat > 1:
                ctx.enter_context(tc.For_i(
                    0, repeat, 1,
                    hint_engines=(mybir.EngineType.PE, mybir.EngineType.SP,
                                  mybir.EngineType.Activation,
                                  mybir.EngineType.DVE, mybir.EngineType.Pool)))
            x_sbs = []
            for b in range(B):
                x_sb = xpool.tile([128, NCH, N], bf16, tag="x_sb")
                nc.sync.dma_start(out=x_sb[:, 0:NCH // 2, :],
                                  in_=xT[:, b, 0:NCH // 2, :])
                nc.sync.dma_start(out=x_sb[:, NCH // 2:, :],
                                  in_=xT[:, b, NCH // 2:, :])
                x_sbs.append(x_sb)
            for b in range(B):
                x_sb = x_sbs[b]
                qT_sb = qkpool.tile([128, N], bf16, tag="qT")
                kT_sb = qkpool.tile([128, N], bf16, tag="kT")
                vp_sb = vpool.tile([128, NTI, HPC, 128], bf16, tag="vp")
                nc.vector.memset(vp_sb[:, :, :, D:], 1.0)

                with tc.tile_pool(name="psA", bufs=2, space="PSUM") as psA, \
                     tc.tile_pool(name="psT", bufs=2, space="PSUM") as psT, \
                     tc.tile_pool(name="stg", bufs=1) as stg, \
                     tc.tile_pool(name="lnp", bufs=2) as lnp, \
                     tc.tile_pool(name="natp", bufs=8) as natp:
                    stage = stg.tile([128, NTI, 6 * D], f32, tag="stage")
                    sqs = stg.tile([128, NTI, 6 * D], f32, tag="sqs")
                    for ti in range(NTI):
                        pq = psA.tile([128, 6 * D], f32, tag="pq")
                        for ci in range(NCH):
                            nc.tensor.matmul(
                                pq,
                                lhsT=x_sb[:, ci, ti * 128:(ti + 1) * 128],
                                rhs=w_sb[:, ci, :],
                                start=(ci == 0),
                                stop=(ci == NCH - 1),
                            )
                        nc.scalar.copy(out=stage[:, ti, :], in_=pq)
                        nc.vector.tensor_tensor(
                            out=sqs[:, ti, :], in0=stage[:, ti, :],
                            in1=stage[:, ti, :], op=mybir.AluOpType.mult)
                    st6v = stage.rearrange("p t (i d) -> p t i d", d=D)
                    sq6v = sqs.rearrange("p t (i d) -> p t i d", d=D)
                    HT = NTI // 2
                    insts = [(0, 0, 0), (1, 1, 0), (3, 0, 1), (4, 1, 1)]
                    for half in range(2):
                     hsl = slice(half * HT, (half + 1) * HT)
                     mean = lnp.tile([128, HT, 6], f32, tag="mean")
                     nc.vector.tensor_reduce(
                        out=mean, in_=st6v[:, hsl], axis=mybir.AxisListType.X,
                        op=mybir.AluOpType.add)
                     nc.vector.tensor_scalar(
                        out=mean, in0=mean, scalar1=1.0 / D, scalar2=None,
                        op0=mybir.AluOpType.mult)
                     var = lnp.tile([128, HT, 6], f32, tag="var")
                     nc.vector.tensor_reduce(
                        out=var, in_=sq6v[:, hsl], axis=mybir.AxisListType.X,
                        op=mybir.AluOpType.add)
                     nc.vector.tensor_scalar(
                        out=var, in0=var, scalar1=1.0 / D, scalar2=None,
                        op0=mybir.AluOpType.mult)
                     m2 = lnp.tile([128, HT, 6], f32, tag="m2")
                     nc.vector.tensor_tensor(
                        out=m2, in0=mean, in1=mean, op=mybir.AluOpType.mult)
                     nc.vector.tensor_tensor(
                        out=var, in0=var, in1=m2, op=mybir.AluOpType.subtract)
                     rstd = lnp.tile([128, HT, 6], f32, tag="rstd")
                     nc.scalar.activation(
                        out=rstd, in_=var,
                        func=mybir.ActivationFunctionType.Sqrt,
                        bias=eps_t, scale=1.0)
                     nc.vector.reciprocal(out=rstd, in_=rstd)
                     if not affine_q:
                        nc.vector.tensor_scalar(
                            out=rstd[:, :, 0:6:3], in0=rstd[:, :, 0:6:3],
                            scalar1=QSCALE, scalar2=None,
                            op0=mybir.AluOpType.mult)
                     for tih in range(HT):
                        ti = half * HT + tih
                        pt = psT.tile([128, 2, 128], bf16, tag="pt")
                        for inst, qk, hh in insts:
                            affine = affine_q if qk == 0 else affine_k
                            nat = natp.tile([128, D], bf16, tag="nat")
                            if affine:
                                natf = natp.tile([128, D], f32, tag="natf")
                                nc.vector.tensor_scalar(
                                    out=natf, in0=st6v[:, ti, inst, :],
                                    scalar1=mean[:, tih, inst:inst + 1],
                                    scalar2=rstd[:, tih, inst:inst + 1],
                                    op0=mybir.AluOpType.subtract,
                                    op1=mybir.AluOpType.mult)
                                r = 0 if qk == 0 else 2
                                natf2 = natp.tile([128, D], f32, tag="natf2")
                                nc.vector.tensor_tensor(
                                    out=natf2, in0=natf, in1=aff_sb[:, r, :],
                                    op=mybir.AluOpType.mult)
                                nc.vector.tensor_tensor(
                                    out=nat, in0=natf2, in1=aff_sb[:, r + 1, :],
                                    op=mybir.AluOpType.add)
                            else:
                                nc.vector.tensor_scalar(
                                    out=nat, in0=st6v[:, ti, inst, :],
                                    scalar1=mean[:, tih, inst:inst + 1],
                                    scalar2=rstd[:, tih, inst:inst + 1],
                                    op0=mybir.AluOpType.subtract,
                                    op1=mybir.AluOpType.mult)
                            nc.tensor.transpose(
                                pt[hh * D:(hh + 1) * D, qk, :], nat, ident)
                        nc.scalar.copy(
                            out=qT_sb[:, ti * 128:(ti + 1) * 128], in_=pt[:, 0, :])
                        nc.scalar.copy(
                            out=kT_sb[:, ti * 128:(ti + 1) * 128], in_=pt[:, 1, :])
                        nc.gpsimd.tensor_copy(
                            out=vp_sb[:, ti, :, 0:D],
                            in_=stage.rearrange("p t (h x) -> p t h x", h=2)
                                [:, ti, :, 2 * D:3 * D])

                with tc.tile_pool(name="psS", bufs=2, space="PSUM") as psS, \
                     tc.tile_pool(name="psAV", bufs=2, space="PSUM") as psAV, \
                     tc.tile_pool(name="psP", bufs=1, space="PSUM") as psP, \
                     tc.tile_pool(name="esp", bufs=2) as esp, \
                     tc.tile_pool(name="otp", bufs=3) as otp, \
                     tc.tile_pool(name="outp", bufs=2) as outp, \
                     tc.tile_pool(name="mskp", bufs=2) as mskp:
                    for ti8 in range(NTI8):
                        i0 = ti8 * QI
                        oTs = []
                        for hh in range(HPC):
                            hs = slice(hh * D, (hh + 1) * D)
                            av = psAV.tile([128, QI], f32, tag="av")
                            for jq in range(4):
                                sT = psS.tile([128, 4, QI], f32, tag="sT")
                                for cj in range(4):
                                    j = jq * 4 + cj
                                    nc.tensor.matmul(
                                        sT[:, cj, :],
                                        lhsT=kT_sb[hs, j * 128:(j + 1) * 128],
                                        rhs=qT_sb[hs, i0:i0 + QI],
                                        start=True, stop=True,
                                    )
                                es = esp.tile([128, 4, QI], bf16, tag="es")
                                nc.scalar.activation(
                                    out=es, in_=sT,
                                    func=mybir.ActivationFunctionType.Exp,
                                )
                                if use_mask:
                                    msk = mskp.tile([128, 4, QI], bf16, tag="msk")
                                    for cj in range(4):
                                        j = jq * 4 + cj
                                        nc.sync.dma_start(
                                            out=msk[:, cj, :],
                                            in_=emaskT[j * 128:(j + 1) * 128,
                                                       i0:i0 + QI],
                                        )
                                    nc.vector.tensor_tensor(
                                        out=es, in0=es, in1=msk,
                                        op=mybir.AluOpType.mult,
                                    )
                                for cj in range(4):
                                    j = jq * 4 + cj
                                    nc.tensor.matmul(
                                        av,
                                        lhsT=vp_sb[:, j, hh, :],
                                        rhs=es[:, cj, :],
                                        start=(j == 0), stop=(j == NTI - 1),
                                    )
                            zr = otp.tile([D, QI], f32, tag="zr")
                            nc.vector.reciprocal(out=zr, in_=av[D:, :])
                            oT = otp.tile([D, QI], bf16, tag="oT")
                            nc.vector.tensor_tensor(
                                out=oT, in0=av[0:D, :], in1=zr,
                                op=mybir.AluOpType.mult,
                            )
                            oTs.append(oT)
                        osb = outp.tile([128, QI // 128, DIM], f32, tag="osb")
                        for sub in range(QI // 128):
                            pp = psP.tile([128, DIM], f32, tag="pp")
                            for nh in range(2):
                                for hh in range(HPC):
                                    nc.tensor.matmul(
                                        pp[:, nh * 512:(nh + 1) * 512],
                                        lhsT=oTs[hh][:, sub * 128:(sub + 1) * 128],
                                        rhs=proj_sb[:, hh, nh * 512:(nh + 1) * 512],
                                        start=(hh == 0), stop=(hh == HPC - 1),
                                    )
                            nc.vector.tensor_copy(out=osb[:, sub, :], in_=pp)
                        ti0 = ti8 * (QI // 128)
                        nc.scalar.dma_start(
                            out=out_p[:, b, ti0:ti0 + QI // 128, :], in_=osb)
    nc.compile()
    return nc


def _prep_legacy(inputs):
    x = np.ascontiguousarray(inputs["x"], dtype=F32)
    qkv_w = np.asarray(inputs["qkv_w"], dtype=F32)
    proj_w = np.asarray(inputs["proj_w"], dtype=F32)
    W_eff = qkv_w.copy()
    for i, (a, bm) in enumerate([("lora_Aq", "lora_Bq"), ("lora_Ak", "lora_Bk"),
                                 ("lora_Av", "lora_Bv")]):
        A = np.asarray(inputs[a], dtype=F32)
        Bm = np.asarray(inputs[bm], dtype=F32)
        W_eff[i * DIM:(i + 1) * DIM] += LORA_SCALE * (A @ Bm).T

    xT_all = np.ascontiguousarray(
        x.transpose(2, 0, 1).reshape(NCH, 128, B, N)
        .transpose(1, 2, 0, 3).astype(BF16))

    use_mask, affine_q, affine_k = _flags(inputs)
    qn_w = np.asarray(inputs["qn_w"], F32); qn_b = np.asarray(inputs["qn_b"], F32)
    kn_w = np.asarray(inputs["kn_w"], F32); kn_b = np.asarray(inputs["kn_b"], F32)
    mask = np.asarray(inputs["attn_mask"], F32)

    common = {"xT": xT_all}
    if affine_q or affine_k:
        aff = np.stack([
            np.broadcast_to(qn_w * QSCALE, (128, D)),
            np.broadcast_to(qn_b * QSCALE, (128, D)),
            np.broadcast_to(kn_w, (128, D)),
            np.broadcast_to(kn_b, (128, D)),
        ]).astype(F32)
        common["lnaff"] = np.ascontiguousarray(aff)
    if use_mask:
        common["emaskT"] = np.ascontiguousarray(
            np.exp(mask[0, 0].T).astype(BF16))

    in_maps = []
    for c in range(NCORES):
        h0 = c * HPC
        blocks = []
        for hh in range(HPC):
            h = h0 + hh
            for part in range(3):
                blocks.append(W_eff[part * DIM + h * D: part * DIM + (h + 1) * D])
        Wlocal = np.concatenate(blocks, axis=0)
        wT_c = np.ascontiguousarray(
            Wlocal.T.reshape(NCH, 128, 6 * D).astype(BF16))
        projT_c = np.ascontiguousarray(np.stack(
            [proj_w[:, (h0 + hh) * D:(h0 + hh + 1) * D].T for hh in range(HPC)],
            axis=1).astype(BF16))
        m = dict(common)
        m["wT"] = wT_c
        m["projT"] = projT_c
        in_maps.append(m)
    return in_maps


def _run_legacy(inputs, key, trace=False):
    in_maps = _prep_legacy(inputs)
    if key not in _prog_cache:
        _prog_cache[key] = _build_legacy(*key)
    nc = _prog_cache[key]
    res = run_bass_kernel_spmd(nc, in_maps, core_ids=list(range(NCORES)),
                               trace=trace)
    acc = np.zeros((128, B, NTI, DIM), dtype=F32)
    for r in res.results:
        acc += r["out_p"]
    out = np.ascontiguousarray(acc.transpose(1, 2, 0, 3).reshape(B, N, DIM))
    out += np.asarray(inputs["proj_b"], F32)
    return out, res


# revision 16
# speedup vs baseline: 2.2527x; 1.0080x over previous
# Trainium2 Bass kernel for nn_CustomAttention (fused qkv + LoRA + per-head
# LayerNorm + softmax attention + output projection).
#
# Sharding: 16 heads split across 8 cores (2 heads/core), both batch elements
# on every core. Each core computes its heads' attention and its partial
# output projection (sum over its heads' columns); the host sums the 8
# partials and adds proj_b. LoRA is folded into the qkv weights on the host:
#   x@W.T + (x@A)@B*s == x@(W + s*(A@B).T).T
# LN mean-removal for q/k is also folded into the weights on the host
# (mean_d q = x @ rowmean(Wq_head)), so the device only needs second moments.
#
# Fast path (no mask, identity LN affine — the graded configuration):
#  - scores and attention@v run in fp8 (e4m3) with MatmulPerfMode.DoubleRow:
#    2 contraction rows per partition -> 2x PE throughput. q/k are stored as
#    [32, 2, N] (d = blk*32 + p), v as [j%128, jtile, head, v|ones].
#  - QSCALE (D^-0.5) is folded into the exp: es = exp(0.125 * s).
#  - rstd = exp(-0.5*ln(ss + 64*eps) + ln(8)) via Act Ln+Exp — both live in
#    the same activation table set as the softmax Exp, so no table reloads.
#  - softmax denominator via stationary [v | ones]: av rows 0-63 hold out^T,
#    rows 64-127 the denominator; one DVE divide produces projected lhsT.
#  - phase interleaving: R0 = qkv+LN+transpose b0; R1 = attention b0
#    interleaved with qkv+LN+transpose b1 (keeps Act busy with exps while PE
#    does b1 qkv); R2 = attention b1 + both batches' output projections.
#    PSUM bank budget (8 banks): R0 psA0(2)+pt0(2); R1 psS(4)+psAV(1)+
#    psA1(1)+pt1(2)=8; R2 psS(4)+psAV(1)+psP(3)=8.
import numpy as np
import ml_dtypes

import concourse.bass as bass
import concourse.bacc as bacc
import concourse.mybir as mybir
from concourse.tile import TileContext
from concourse.masks import make_identity
from concourse.bass_utils import run_bass_kernel_spmd

BF16 = ml_dtypes.bfloat16
F32 = np.float32

B, N, DIM, H, R = 2, 2048, 1024, 16, 8
D = DIM // H              # 64
NCORES = 8
HPC = H // NCORES         # 2 heads per core
ALPHA = 8.0
LORA_SCALE = ALPHA / R
EPS = 1e-5
QSCALE = float(D) ** -0.5  # 0.125

NCH = DIM // 128          # 8 contraction chunks of 128
NTI = N // 128            # 16 row tiles of 128
NTI8 = N // 256           # 8 i-tiles of 256
QI = 256                  # query-tile width (free dim of score matmuls)

_prog_cache: dict = {}


def _build_fast(repeat: int = 1):
    nc = bacc.Bacc("TRN2", target_bir_lowering=False)
    f32 = mybir.dt.float32
    bf16 = mybir.dt.bfloat16
    fp8 = mybir.dt.float8e4
    DR = mybir.MatmulPerfMode.DoubleRow
    Exp = mybir.ActivationFunctionType.Exp
    Ln = mybir.ActivationFunctionType.Ln

    xT = nc.dram_tensor("xT", [128, B, NCH, N], bf16, kind="ExternalInput")
    wT = nc.dram_tensor("wT", [NCH, 128, 6 * D], bf16, kind="ExternalInput")
    projT = nc.dram_tensor("projT", [128, DIM], bf16, kind="ExternalInput")
    out_p = nc.dram_tensor("out_p", [128, B, NTI, DIM], bf16, kind="ExternalOutput")

    with TileContext(nc) as tc:
        import contextlib
        with contextlib.ExitStack() as ctx:
            const = ctx.enter_context(tc.tile_pool(name="const", bufs=1))
            ident8 = const.tile([128, 128], fp8)
            make_identity(nc, ident8)
            beps = const.tile([128, 1], f32)
            nc.vector.memset(beps, 64.0 * EPS)
            bln8 = const.tile([128, 1], f32)
            nc.vector.memset(bln8, float(np.log(8.0)))

            persist = ctx.enter_context(tc.tile_pool(name="persist", bufs=1))
            w_sb = persist.tile([128, NCH, 6 * D], bf16)
            nc.sync.dma_start(out=w_sb, in_=wT.rearrange("ci cm w -> cm ci w"))
            proj_sb = persist.tile([128, DIM], bf16)
            nc.sync.dma_start(out=proj_sb, in_=projT[:, :])

            xpool = ctx.enter_context(tc.tile_pool(name="xpool", bufs=2))
            qk8p = ctx.enter_context(tc.tile_pool(name="qk8p", bufs=2))
            v8p = ctx.enter_context(tc.tile_pool(name="v8p", bufs=2))
            oTp = ctx.enter_context(tc.tile_pool(name="oTp", bufs=2))
            stg = ctx.enter_context(tc.tile_pool(name="stg", bufs=18))
            sqp = ctx.enter_context(tc.tile_pool(name="sqp", bufs=3))
            lnp = ctx.enter_context(tc.tile_pool(name="lnp", bufs=2))
            natp = ctx.enter_context(tc.tile_pool(name="natp", bufs=6))
            esp = ctx.enter_context(tc.tile_pool(name="esp", bufs=3))
            osbp = ctx.enter_context(tc.tile_pool(name="osbp", bufs=4))

            if repeat > 1:
                ctx.enter_context(tc.For_i(
                    0, repeat, 1,
                    hint_engines=(mybir.EngineType.PE, mybir.EngineType.SP,
                                  mybir.EngineType.Activation,
                                  mybir.EngineType.DVE, mybir.EngineType.Pool)))

            # ---- per-iteration activations -------------------------------
            x_sbs, qT8s, kT8s, vp8s, oT2s = [], [], [], [], []
            for b in range(B):
                x_sb = xpool.tile([128, NCH, N], bf16, tag="x_sb")
                nc.sync.dma_start(out=x_sb[:, 0:NCH // 2, :],
                                  in_=xT[:, b, 0:NCH // 2, :])
                nc.sync.dma_start(out=x_sb[:, NCH // 2:, :],
                                  in_=xT[:, b, NCH // 2:, :])
                x_sbs.append(x_sb)
            for b in range(B):
                qT8 = qk8p.tile([64, 2, N], fp8, tag="qT8")
                kT8 = qk8p.tile([64, 2, N], fp8, tag="kT8")
                qT8s.append(qT8)
                kT8s.append(kT8)
                vp8 = v8p.tile([128, NTI, HPC, 128], fp8, tag="vp8")
                nc.vector.memset(vp8[:, :, :, D:], 1.0)
                vp8s.append(vp8)
                oT2 = oTp.tile([128, NTI8, QI], bf16, tag="oT2")
                oT2s.append(oT2)

            # state carried across a_qkv / rstd_calc / a_fin
            a_state: dict = {}

            def a_qkv(b, ti, psA):
                """qkv matmuls + LN second moments for row-tile ti."""
                x_sb = x_sbs[b]
                pq = psA.tile([128, 6 * D], f32, tag="pq")
                for ci in range(NCH):
                    nc.tensor.matmul(
                        pq,
                        lhsT=x_sb[:, ci, ti * 128:(ti + 1) * 128],
                        rhs=w_sb[:, ci, :],
                        start=(ci == 0),
                        stop=(ci == NCH - 1),
                    )
                st = stg.tile([128, 6, D], bf16, tag="st")
                nc.gpsimd.tensor_copy(
                    out=st, in_=pq.rearrange("p (i d) -> p i d", d=D))
                a_state[(b, "st", ti)] = st
                # squares of q/k instances only: [128, head(2), qk(2), D]
                qkv_view = st.rearrange("p i d -> p (i d)") \
                    .rearrange("p (h r) -> p h r", h=2) \
                    .rearrange("p h (i d) -> p h i d", d=D)[:, :, 0:2, :]
                sq = sqp.tile([128, 2, 2, D], f32, tag="sq")
                nc.vector.tensor_tensor(
                    out=sq, in0=qkv_view, in1=qkv_view,
                    op=mybir.AluOpType.mult)
                if ti == 0:
                    ss = lnp.tile([128, NTI, 4], f32, tag="ssall")
                    a_state[(b, "ss")] = ss
                else:
                    ss = a_state[(b, "ss")]
                nc.vector.tensor_reduce(
                    out=ss[:, ti, :],
                    in_=sq, axis=mybir.AxisListType.X,
                    op=mybir.AluOpType.add)

            def rstd_calc(b, half=None):
                """rsqrt(ss/64+eps) = exp(-0.5*ln(ss + 64 eps) + ln(8)).
                Clustered so the Ln<->Exp table swap happens once, not per
                tile."""
                ss = a_state[(b, "ss")]
                sl = slice(0, NTI) if half is None else \
                    slice(half * (NTI // 2), (half + 1) * (NTI // 2))
                nt = NTI if half is None else NTI // 2
                if (b, "rstd") not in a_state:
                    rstdall = lnp.tile([128, NTI, 4], f32, tag="rstdall")
                    a_state[(b, "rstd")] = rstdall
                rstd = a_state[(b, "rstd")]
                tln = lnp.tile([128, nt * 4], f32, tag="tln")
                nc.scalar.activation(
                    out=tln, in_=ss[:, sl, :].rearrange("p a b -> p (a b)"),
                    func=Ln, bias=beps, scale=1.0)
                nc.scalar.activation(
                    out=rstd[:, sl, :].rearrange("p a b -> p (a b)"),
                    in_=tln, func=Exp, scale=-0.5, bias=bln8)

            def a_fin(b, ti, pt):
                """normalize -> fp8 transpose -> qT8/kT8/vp8 copies."""
                st_h = a_state.pop((b, "st", ti))
                rstd = a_state[(b, "rstd")]
                ptt = pt.tile([64, 2, 2, 128], fp8, tag="pt")
                for idx, (inst, qki, hh) in enumerate(
                        [(0, 0, 0), (1, 1, 0), (3, 0, 1), (4, 1, 1)]):
                    nat = natp.tile([128, D], fp8, tag="nat")
                    nc.vector.tensor_scalar(
                        out=nat, in0=st_h[:, inst, :],
                        scalar1=rstd[:, ti, idx:idx + 1],
                        scalar2=None, op0=mybir.AluOpType.mult)
                    for blk in range(2):
                        nc.tensor.transpose(
                            ptt[hh * 32:(hh + 1) * 32, qki, blk, :],
                            nat[:, blk * 32:(blk + 1) * 32], ident8)
                nc.gpsimd.tensor_copy(
                    out=qT8s[b][:, :, ti * 128:(ti + 1) * 128],
                    in_=ptt[:, 0, :, :])
                nc.gpsimd.tensor_copy(
                    out=kT8s[b][:, :, ti * 128:(ti + 1) * 128],
                    in_=ptt[:, 1, :, :])
                nc.gpsimd.tensor_copy(
                    out=vp8s[b][:, ti, :, 0:D],
                    in_=st_h[:, 2:6:3, :])

            def b_block(b, ti8, hh, psS, psAV):
                i0 = ti8 * QI
                av = psAV.tile([128, QI], f32, tag="av")
                for g in range(4):
                    sT = psS.tile([128, 4, QI], f32, tag="sT")
                    for cj in range(4):
                        j = g * 4 + cj
                        nc.tensor.matmul(
                            sT[:, cj, :],
                            lhsT=kT8s[b][hh * 32:(hh + 1) * 32, :,
                                         j * 128:(j + 1) * 128],
                            rhs=qT8s[b][hh * 32:(hh + 1) * 32, :, i0:i0 + QI],
                            start=True, stop=True, perf_mode=DR)
                    es = esp.tile([128, 4, QI], fp8, tag="es")
                    nc.scalar.activation(out=es, in_=sT, func=Exp,
                                         scale=QSCALE)
                    for q2 in range(2):
                        j0 = g * 4 + 2 * q2
                        nc.tensor.matmul(
                            av,
                            lhsT=vp8s[b][:, j0:j0 + 2, hh, :],
                            rhs=es[:, 2 * q2:2 * q2 + 2, :],
                            start=(g == 0 and q2 == 0),
                            stop=(g == 3 and q2 == 1), perf_mode=DR)
                nc.vector.tensor_tensor(
                    out=oT2s[b][hh * D:(hh + 1) * D, ti8, :],
                    in0=av[0:D, :], in1=av[D:, :],
                    op=mybir.AluOpType.divide)

            def proj_chunk(b, ti8, psP, alt):
                for sub in range(2):
                    for nh in range(2):
                        pp = psP.tile([128, 512], f32, tag="pp")
                        nc.tensor.matmul(
                            pp,
                            lhsT=oT2s[b][:, ti8, sub * 128:(sub + 1) * 128],
                            rhs=proj_sb[:, nh * 512:(nh + 1) * 512],
                            start=True, stop=True)
                        osb = osbp.tile([128, 512], bf16, tag="osb")
                        eng = nc.vector if (alt + sub * 2 + nh) % 2 == 0 \
                            else nc.gpsimd
                        eng.tensor_copy(out=osb, in_=pp)
                        nc.sync.dma_start(
                            out=out_p[:, b, ti8 * 2 + sub,
                                      nh * 512:(nh + 1) * 512],
                            in_=osb)

            # Region structure (PSUM banks in [] out of 8):
            #  R0a: qkv+stats b0 under psA0[2]+pt[2]
            #  R0b: fins b0 + first b0 attention blocks under pt[2]+psS[4]+
            #       psAV[1]
            #  R1:  rest of b0 attention + full phase A of b1 (+psA1[1] = 8)
            #  R2:  b1 attention + both projections (psP[3])
            ptp_cm = tc.tile_pool(name="ptp", bufs=2, space="PSUM",
                                  side="right")
            psA0_cm = tc.tile_pool(name="psA0", bufs=2, space="PSUM",
                                   side="right")
            psS_cm = tc.tile_pool(name="psS", bufs=2, space="PSUM",
                                  side="left")
            psAV_cm = tc.tile_pool(name="psAV", bufs=1, space="PSUM",
                                   side="left")
            psA1_cm = tc.tile_pool(name="psA1", bufs=1, space="PSUM",
                                   side="right")
            psP_cm = tc.tile_pool(name="psP", bufs=3, space="PSUM",
                                  side="left")
            ptp = ptp_cm.__enter__()
            psA0 = psA0_cm.__enter__()
            for ti in range(NTI // 2):
                a_qkv(0, ti, psA0)
            rstd_calc(0, half=0)
            for ti in range(NTI // 2, NTI):
                a_qkv(0, ti, psA0)
                a_fin(0, ti - NTI // 2, ptp)
            rstd_calc(0, half=1)
            psA0_cm.__exit__(None, None, None)
            psS = psS_cm.__enter__()
            psAV = psAV_cm.__enter__()
            for ti in range(NTI // 2, NTI):
                a_fin(0, ti, ptp)
            for k in range(4):
                b_block(0, k // 2, k % 2, psS, psAV)
            psA1 = psA1_cm.__enter__()
            for k in range(4, 16):
                b_block(0, k // 2, k % 2, psS, psAV)
                if k < 12:
                    a_qkv(1, 2 * (k - 4), psA1)
                    a_qkv(1, 2 * (k - 4) + 1, psA1)
                else:
                    if k == 12:
                        rstd_calc(1)
                    for f in range(4):
                        a_fin(1, 4 * (k - 12) + f, ptp)
            psA1_cm.__exit__(None, None, None)
            ptp_cm.__exit__(None, None, None)
            psP = psP_cm.__enter__()
            for k in range(16):
                b_block(1, k // 2, k % 2, psS, psAV)
                if k % 2 == 0:
                    proj_chunk(0, k // 2, psP, alt=0)
                elif k >= 3:
                    proj_chunk(1, (k - 3) // 2, psP, alt=1)
            proj_chunk(1, 7, psP, alt=1)
            psP_cm.__exit__(None, None, None)
            psAV_cm.__exit__(None, None, None)
            psS_cm.__exit__(None, None, None)
    nc.compile()
    return nc


def _prep_fast(inputs):
    x = np.ascontiguousarray(inputs["x"], dtype=F32)
    qkv_w = np.asarray(inputs["qkv_w"], dtype=F32)
    proj_w = np.asarray(inputs["proj_w"], dtype=F32)
    W_eff = qkv_w.copy()
    for i, (a, bm) in enumerate([("lora_Aq", "lora_Bq"), ("lora_Ak", "lora_Bk"),
                                 ("lora_Av", "lora_Bv")]):
        A = np.asarray(inputs[a], dtype=F32)
        Bm = np.asarray(inputs[bm], dtype=F32)
        W_eff[i * DIM:(i + 1) * DIM] += LORA_SCALE * (A @ Bm).T
    # fold LN mean removal into q/k weights (per head row-block)
    for part in range(2):
        for h in range(H):
            blk = W_eff[part * DIM + h * D: part * DIM + (h + 1) * D]
            blk -= blk.mean(axis=0, keepdims=True)

    xT_all = np.ascontiguousarray(
        x.transpose(2, 0, 1).reshape(NCH, 128, B, N)
        .transpose(1, 2, 0, 3).astype(BF16))

    in_maps = []
    for c in range(NCORES):
        h0 = c * HPC
        blocks = []
        for hh in range(HPC):
            h = h0 + hh
            for part in range(3):  # q, k, v
                blocks.append(W_eff[part * DIM + h * D: part * DIM + (h + 1) * D])
        Wlocal = np.concatenate(blocks, axis=0)          # [384, 1024]
        wT_c = np.ascontiguousarray(
            Wlocal.T.reshape(NCH, 128, 6 * D).astype(BF16))
        projT_c = np.ascontiguousarray(np.concatenate(
            [proj_w[:, (h0 + hh) * D:(h0 + hh + 1) * D].T for hh in range(HPC)],
            axis=0).astype(BF16))                        # [128, 1024]
        in_maps.append({"xT": xT_all, "wT": wT_c, "projT": projT_c})
    return in_maps


def _flags(inputs):
    qn_w = np.asarray(inputs["qn_w"], F32); qn_b = np.asarray(inputs["qn_b"], F32)
    kn_w = np.asarray(inputs["kn_w"], F32); kn_b = np.asarray(inputs["kn_b"], F32)
    affine_q = not (np.all(qn_w == 1.0) and np.all(qn_b == 0.0))
    affine_k = not (np.all(kn_w == 1.0) and np.all(kn_b == 0.0))
    mask = np.asarray(inputs["attn_mask"], F32)
    use_mask = bool(np.any(mask))
    return use_mask, affine_q, affine_k


def _run(inputs, trace=False):
    key = _flags(inputs)
    if any(key):
        return _run_legacy(inputs, key, trace)
    if ("fast", 1) not in _prog_cache:
        _prog_cache[("fast", 1)] = _build_fast(repeat=1)
    nc = _prog_cache[("fast", 1)]
    in_maps = _prep_fast(inputs)
    res = run_bass_kernel_spmd(nc, in_maps, core_ids=list(range(NCORES)),
                               trace=trace)
    acc = np.zeros((128, B, NTI, DIM), dtype=F32)
    for r in res.results:
        acc += np.asarray(r["out_p"], dtype=F32)
    out = np.ascontiguousarray(acc.transpose(1, 2, 0, 3).reshape(B, N, DIM))
    out += np.asarray(inputs["proj_b"], F32)
    return out, res


def kernel(**inputs) -> np.ndarray:
    out, _ = _run(inputs)
    return out


# ======================= legacy path (mask / affine LN) ====================

def _build_legacy(use_mask: bool, affine_q: bool, affine_k: bool,
                  repeat: int = 1):
    nc = bacc.Bacc("TRN2", target_bir_lowering=False)
    f32 = mybir.dt.float32
    bf16 = mybir.dt.bfloat16

    xT = nc.dram_tensor("xT", [128, B, NCH, N], bf16, kind="ExternalInput")
    wT = nc.dram_tensor("wT", [NCH, 128, 6 * D], bf16, kind="ExternalInput")
    projT = nc.dram_tensor("projT", [D, HPC, DIM], bf16, kind="ExternalInput")
    out_p = nc.dram_tensor("out_p", [128, B, NTI, DIM], f32, kind="ExternalOutput")
    if affine_q or affine_k:
        lnaff = nc.dram_tensor("lnaff", [4, 128, D], f32, kind="ExternalInput")
    if use_mask:
        emaskT = nc.dram_tensor("emaskT", [N, N], bf16, kind="ExternalInput")

    with TileContext(nc) as tc:
        import contextlib
        with contextlib.ExitStack() as ctx:
            const = ctx.enter_context(tc.tile_pool(name="const", bufs=1))
            ident = const.tile([128, 128], bf16)
            make_identity(nc, ident)
            eps_t = const.tile([128, 1], f32)
            nc.vector.memset(eps_t, EPS)

            persist = ctx.enter_context(tc.tile_pool(name="persist", bufs=1))
            w_sb = persist.tile([128, NCH, 6 * D], bf16)
            nc.sync.dma_start(out=w_sb, in_=wT.rearrange("ci cm w -> cm ci w"))
            proj_sb = persist.tile([D, HPC, DIM], bf16)
            nc.sync.dma_start(out=proj_sb, in_=projT[:, :, :])
            if affine_q or affine_k:
                aff_sb = persist.tile([128, 4, D], f32)
                nc.sync.dma_start(out=aff_sb, in_=lnaff.rearrange("r p d -> p r d"))

            xpool = ctx.enter_context(tc.tile_pool(name="xpool", bufs=2))
            qkpool = ctx.enter_context(tc.tile_pool(name="qkpool", bufs=2))
            vpool = ctx.enter_context(tc.tile_pool(name="vpool", bufs=2))

            if repeat > 1:
                ctx.enter_context(tc.For_i(
                    0, repeat, 1,
                    hint_engines=(mybir.EngineType.PE, mybir.EngineType.SP,
                                  mybir.EngineType.Activation,
                                  mybir.EngineType.DVE, mybir.EngineType.Pool)))
            x_sbs = []
            for b in range(B):
                x_sb = xpool.tile([128, NCH, N], bf16, tag="x_sb")
                nc.sync.dma_start(out=x_sb[:, 0:NCH // 2, :],
                                  in_=xT[:, b, 0:NCH // 2, :])
                nc.sync.dma_start(out=x_sb[:, NCH // 2:, :],
                                  in_=xT[:, b, NCH // 2:, :])
                x_sbs.append(x_sb)
            for b in range(B):
                x_sb = x_sbs[b]
                qT_sb = qkpool.tile([128, N], bf16, tag="qT")
                kT_sb = qkpool.tile([128, N], bf16, tag="kT")
                vp_sb = vpool.tile([128, NTI, HPC, 128], bf16, tag="vp")
                nc.vector.memset(vp_sb[:, :, :, D:], 1.0)

                with tc.tile_pool(name="psA", bufs=2, space="PSUM") as psA, \
                     tc.tile_pool(name="psT", bufs=2, space="PSUM") as psT, \
                     tc.tile_pool(name="stg", bufs=1) as stg, \
                     tc.tile_pool(name="lnp", bufs=2) as lnp, \
                     tc.tile_pool(name="natp", bufs=8) as natp:
                    stage = stg.tile([128, NTI, 6 * D], f32, tag="stage")
                    sqs = stg.tile([128, NTI, 6 * D], f32, tag="sqs")
                    for ti in range(NTI):
                        pq = psA.tile([128, 6 * D], f32, tag="pq")
                        for ci in range(NCH):
                            nc.tensor.matmul(
                                pq,
                                lhsT=x_sb[:, ci, ti * 128:(ti + 1) * 128],
                                rhs=w_sb[:, ci, :],
                                start=(ci == 0),
                                stop=(ci == NCH - 1),
                            )
                        nc.scalar.copy(out=stage[:, ti, :], in_=pq)
                        nc.vector.tensor_tensor(
                            out=sqs[:, ti, :], in0=stage[:, ti, :],
                            in1=stage[:, ti, :], op=mybir.AluOpType.mult)
                    st6v = stage.rearrange("p t (i d) -> p t i d", d=D)
                    sq6v = sqs.rearrange("p t (i d) -> p t i d", d=D)
                    HT = NTI // 2
                    insts = [(0, 0, 0), (1, 1, 0), (3, 0, 1), (4, 1, 1)]
                    for half in range(2):
                     hsl = slice(half * HT, (half + 1) * HT)
                     mean = lnp.tile([128, HT, 6], f32, tag="mean")
                     nc.vector.tensor_reduce(
                        out=mean, in_=st6v[:, hsl], axis=mybir.AxisListType.X,
                        op=mybir.AluOpType.add)
                     nc.vector.tensor_scalar(
                        out=mean, in0=mean, scalar1=1.0 / D, scalar2=None,
                        op0=mybir.AluOpType.mult)
                     var = lnp.tile([128, HT, 6], f32, tag="var")
                     nc.vector.tensor_reduce(
                        out=var, in_=sq6v[:, hsl], axis=mybir.AxisListType.X,
                        op=mybir.AluOpType.add)
                     nc.vector.tensor_scalar(
                        out=var, in0=var, scalar1=1.0 / D, scalar2=None,
                        op0=mybir.AluOpType.mult)
                     m2 = lnp.tile([128, HT, 6], f32, tag="m2")
                     nc.vector.tensor_tensor(
                        out=m2, in0=mean, in1=mean, op=mybir.AluOpType.mult)
                     nc.vector.tensor_tensor(
                        out=var, in0=var, in1=m2, op=mybir.AluOpType.subtract)
                     rstd = lnp.tile([128, HT, 6], f32, tag="rstd")
                     nc.scalar.activation(
                        out=rstd, in_=var,
                        func=mybir.ActivationFunctionType.Sqrt,
                        bias=eps_t, scale=1.0)
                     nc.vector.reciprocal(out=rstd, in_=rstd)
                     if not affine_q:
                        nc.vector.tensor_scalar(
                            out=rstd[:, :, 0:6:3], in0=rstd[:, :, 0:6:3],
                            scalar1=QSCALE, scalar2=None,
                            op0=mybir.AluOpType.mult)
                     for tih in range(HT):
                        ti = half * HT + tih
                        pt = psT.tile([128, 2, 128], bf16, tag="pt")
                        for inst, qk, hh in insts:
                            affine = affine_q if qk == 0 else affine_k
                            nat = natp.tile([128, D], bf16, tag="nat")
                            if affine:
                                natf = natp.tile([128, D], f32, tag="natf")
                                nc.vector.tensor_scalar(
                                    out=natf, in0=st6v[:, ti, inst, :],
                                    scalar1=mean[:, tih, inst:inst + 1],
                                    scalar2=rstd[:, tih, inst:inst + 1],
                                    op0=mybir.AluOpType.subtract,
                                    op1=mybir.AluOpType.mult)
                                r = 0 if qk == 0 else 2
                                natf2 = natp.tile([128, D], f32, tag="natf2")
                                nc.vector.tensor_tensor(
                                    out=natf2, in0=natf, in1=aff_sb[:, r, :],
                                    op=mybir.AluOpType.mult)
                                nc.vector.tensor_tensor(
                                    out=nat, in0=natf2, in1=aff_sb[:, r + 1, :],
                                    op=mybir.AluOpType.add)
                            else:
                                nc.vector.tensor_scalar(
                                    out=nat, in0=st6v[:, ti, inst, :],
                                    scalar1=mean[:, tih, inst:inst + 1],
                                    scalar2=rstd[:, tih, inst:inst + 1],
                                    op0=mybir.AluOpType.subtract,
                                    op1=mybir.AluOpType.mult)
                            nc.tensor.transpose(
                                pt[hh * D:(hh + 1) * D, qk, :], nat, ident)
                        nc.scalar.copy(
                            out=qT_sb[:, ti * 128:(ti + 1) * 128], in_=pt[:, 0, :])
                        nc.scalar.copy(
                            out=kT_sb[:, ti * 128:(ti + 1) * 128], in_=pt[:, 1, :])
                        nc.gpsimd.tensor_copy(
                            out=vp_sb[:, ti, :, 0:D],
                            in_=stage.rearrange("p t (h x) -> p t h x", h=2)
                                [:, ti, :, 2 * D:3 * D])

                with tc.tile_pool(name="psS", bufs=2, space="PSUM") as psS, \
                     tc.tile_pool(name="psAV", bufs=2, space="PSUM") as psAV, \
                     tc.tile_pool(name="psP", bufs=1, space="PSUM") as psP, \
                     tc.tile_pool(name="esp", bufs=2) as esp, \
                     tc.tile_pool(name="otp", bufs=3) as otp, \
                     tc.tile_pool(name="outp", bufs=2) as outp, \
                     tc.tile_pool(name="mskp", bufs=2) as mskp:
                    for ti8 in range(NTI8):
                        i0 = ti8 * QI
                        oTs = []
                        for hh in range(HPC):
                            hs = slice(hh * D, (hh + 1) * D)
                            av = psAV.tile([128, QI], f32, tag="av")
                            for jq in range(4):
                                sT = psS.tile([128, 4, QI], f32, tag="sT")
                                for cj in range(4):
                                    j = jq * 4 + cj
                                    nc.tensor.matmul(
                                        sT[:, cj, :],
                                        lhsT=kT_sb[hs, j * 128:(j + 1) * 128],
                                        rhs=qT_sb[hs, i0:i0 + QI],
                                        start=True, stop=True,
                                    )
                                es = esp.tile([128, 4, QI], bf16, tag="es")
                                nc.scalar.activation(
                                    out=es, in_=sT,
                                    func=mybir.ActivationFunctionType.Exp,
                                )
                                if use_mask:
                                    msk = mskp.tile([128, 4, QI], bf16, tag="msk")
                                    for cj in range(4):
                                        j = jq * 4 + cj
                                        nc.sync.dma_start(
                                            out=msk[:, cj, :],
                                            in_=emaskT[j * 128:(j + 1) * 128,
                                                       i0:i0 + QI],
                                        )
                                    nc.vector.tensor_tensor(
                                        out=es, in0=es, in1=msk,
                                        op=mybir.AluOpType.mult,
                                    )
                                for cj in range(4):
                                    j = jq * 4 + cj
                                    nc.tensor.matmul(
                                        av,
                                        lhsT=vp_sb[:, j, hh, :],
                                        rhs=es[:, cj, :],
                                        start=(j == 0), stop=(j == NTI - 1),
                                    )
                            zr = otp.tile([D, QI], f32, tag="zr")
                            nc.vector.reciprocal(out=zr, in_=av[D:, :])
                            oT = otp.tile([D, QI], bf16, tag="oT")
                            nc.vector.tensor_tensor(
                                out=oT, in0=av[0:D, :], in1=zr,
                                op=mybir.AluOpType.mult,
                            )
                            oTs.append(oT)
                        osb = outp.tile([128, QI // 128, DIM], f32, tag="osb")
                        for sub in range(QI // 128):
                            pp = psP.tile([128, DIM], f32, tag="pp")
                            for nh in range(2):
                                for hh in range(HPC):
                                    nc.tensor.matmul(
                                        pp[:, nh * 512:(nh + 1) * 512],
                                        lhsT=oTs[hh][:, sub * 128:(sub + 1) * 128],
                                        rhs=proj_sb[:, hh, nh * 512:(nh + 1) * 512],
                                        start=(hh == 0), stop=(hh == HPC - 1),
                                    )
                            nc.vector.tensor_copy(out=osb[:, sub, :], in_=pp)
                        ti0 = ti8 * (QI // 128)
                        nc.scalar.dma_start(
                            out=out_p[:, b, ti0:ti0 + QI // 128, :], in_=osb)
    nc.compile()
    return nc


def _prep_legacy(inputs):
    x = np.ascontiguousarray(inputs["x"], dtype=F32)
    qkv_w = np.asarray(inputs["qkv_w"], dtype=F32)
    proj_w = np.asarray(inputs["proj_w"], dtype=F32)
    W_eff = qkv_w.copy()
    for i, (a, bm) in enumerate([("lora_Aq", "lora_Bq"), ("lora_Ak", "lora_Bk"),
                                 ("lora_Av", "lora_Bv")]):
        A = np.asarray(inputs[a], dtype=F32)
        Bm = np.asarray(inputs[bm], dtype=F32)
        W_eff[i * DIM:(i + 1) * DIM] += LORA_SCALE * (A @ Bm).T

    xT_all = np.ascontiguousarray(
        x.transpose(2, 0, 1).reshape(NCH, 128, B, N)
        .transpose(1, 2, 0, 3).astype(BF16))

    use_mask, affine_q, affine_k = _flags(inputs)
    qn_w = np.asarray(inputs["qn_w"], F32); qn_b = np.asarray(inputs["qn_b"], F32)
    kn_w = np.asarray(inputs["kn_w"], F32); kn_b = np.asarray(inputs["kn_b"], F32)
    mask = np.asarray(inputs["attn_mask"], F32)

    common = {"xT": xT_all}
    if affine_q or affine_k:
        aff = np.stack([
            np.broadcast_to(qn_w * QSCALE, (128, D)),
            np.broadcast_to(qn_b * QSCALE, (128, D)),
            np.broadcast_to(kn_w, (128, D)),
            np.broadcast_to(kn_b, (128, D)),
        ]).astype(F32)
        common["lnaff"] = np.ascontiguousarray(aff)
    if use_mask:
        common["emaskT"] = np.ascontiguousarray(
            np.exp(mask[0, 0].T).astype(BF16))

    in_maps = []
    for c in range(NCORES):
        h0 = c * HPC
        blocks = []
        for hh in range(HPC):
            h = h0 + hh
            for part in range(3):
                blocks.append(W_eff[part * DIM + h * D: part * DIM + (h + 1) * D])
        Wlocal = np.concatenate(blocks, axis=0)
        wT_c = np.ascontiguousarray(
            Wlocal.T.reshape(NCH, 128, 6 * D).astype(BF16))
        projT_c = np.ascontiguousarray(np.stack(
            [proj_w[:, (h0 + hh) * D:(h0 + hh + 1) * D].T for hh in range(HPC)],
            axis=1).astype(BF16))
        m = dict(common)
        m["wT"] = wT_c
        m["projT"] = projT_c
        in_maps.append(m)
    return in_maps


def _run_legacy(inputs, key, trace=False):
    in_maps = _prep_legacy(inputs)
    if key not in _prog_cache:
        _prog_cache[key] = _build_legacy(*key)
    nc = _prog_cache[key]
    res = run_bass_kernel_spmd(nc, in_maps, core_ids=list(range(NCORES)),
                               trace=trace)
    acc = np.zeros((128, B, NTI, DIM), dtype=F32)
    for r in res.results:
        acc += r["out_p"]
    out = np.ascontiguousarray(acc.transpose(1, 2, 0, 3).reshape(B, N, DIM))
    out += np.asarray(inputs["proj_b"], F32)
    return out, res
